# revision 31
# baseline (speedup 1.0000x reference)
"""Trainium2 Bass kernel for nn_Attention_17532056502607.

Multi-head self-attention (B=8, N=48*48=2304 tokens, C=384, 8 heads of 48):
    q = x @ q_w.T + q_b ; k,v = x @ kv_w.T + kv_b
    out = softmax(q k^T / sqrt(48)) v ; y = out @ proj_w.T + proj_b

Sharding: data-parallel, one batch element per NeuronCore (8 cores).

Per-core algorithm (all in "S^T layout", keys on partitions — no transposes):
  - host supplies xT = x_b^T [C, N] and head-PAIR-packed weights: heads 2p /
    2p+1 of a pair live at partition rows 0-47 / 64-111, so two K=48 matmuls
    run concurrently in the PE array (row/col 32-tiles).
  - qT/kT [C_pair, N] = wT-pair @ xT          (PE, K=C=384)
  - v    [N, 8*49]    = x @ wv + rank-1 bias matmul; each head's V block is
    [ones | v0..v47], so attn@V also accumulates the softmax denominator at
    a 32-aligned output partition (0 / 64).
  - S^T  [keys, q]    = kT-tile.T @ qT        (K=48, row-packed head pairs)
  - expS = exp(scale * S^T)                   (ACT, reads PSUM, writes SBUF)
  - outT [49x2, q]   += (1|v).T @ expS        (K=128 keys, col strips 0/64)
  - normalize: drain to SBUF, exact reciprocal of rows 0/64, rank-1 selector
    matmul broadcasts the recips, DVE multiply.
  - y    [N, C]       = sum_pairs outT-pair.T @ projw-pair + bias, with K=113
    spanning both head blocks and zero weight rows under the denominators.

Matmul dtypes default to float32r for x->q/k/v and the output projection and
bf16 for the attention core (rel err ~2.7e-3 vs the fp32 reference; set
ATTN_MM_DT=float32 for exact-but-slow).
"""

import os
import sys

import numpy as np

for _p in ("/opt/trn_rl_repo",):
    if _p not in sys.path:
        sys.path.append(_p)

import concourse.bass as bass  # noqa: E402
import concourse.tile as tile  # noqa: E402
from concourse import bacc, mybir  # noqa: E402
from concourse.bass_utils import run_bass_kernel_spmd  # noqa: E402

# ---------------------------------------------------------------- constants
B = 8
HH = 48
WW = 48
C = 384
N = HH * WW  # 2304
NH = 8
HD = 48
PAIRS = NH // 2  # 4
P = 128
NT = N // P  # 18 token tiles
KTC = C // P  # 3 contraction tiles over C
SCALE = float(HD) ** -0.5
VW = NH * (HD + 1)  # 392: v with a ones column per head
CHUNKS = [(0, 512), (512, 512), (1024, 512), (1536, 512), (2048, 256)]

F32 = mybir.dt.float32
# Matmul dtype for all SBUF operands. float32 = exact but 4 cyc/row on PE;
# float32r = same bits, reduced-precision single-pass matmul (1 cyc/row for
# moving dim >= 256) but cannot write PSUM at partition base 64; bfloat16
# halves SBUF/DMA and enables FWL.
MM_DT = getattr(mybir.dt, os.environ.get("ATTN_MM_DT", "bfloat16"))
# attn@V runs in bf16 when MM_DT is float32r (f32r matmuls cannot col-tile to
# partition base 64; bf16 error here is averaged over the 2304-key softmax).
AV_DT = (
    mybir.dt.bfloat16
    if MM_DT == mybir.dt.float32r
    else getattr(mybir.dt, os.environ.get("ATTN_AV_DT", MM_DT.value))
)

# S^T (q@k) operand dtype. bf16 emits separate LDWEIGHTS instructions that
# overlap prior matmuls in the other row group (fp32r self-loads weights
# serially); the softmax averages away the extra rounding (+6% rel err).
ST_DT = getattr(
    mybir.dt,
    os.environ.get(
        "ATTN_ST_DT",
        "bfloat16" if MM_DT == mybir.dt.float32r else MM_DT.value,
    ),
)

# broadcast-matmul operand dtype: f32r is 4x faster on PE and legal at dst
# base 0; producers must write f32r-typed outputs (verifier checks rounding)
BC_DT = mybir.dt.float32r if MM_DT != mybir.dt.float32 else F32

_EXP = mybir.ActivationFunctionType.Exp


def _emit(tc: tile.TileContext, d: dict, ctx):
    nc = tc.nc

    persist = ctx.enter_context(tc.tile_pool(name="persist", bufs=1))
    v_sb = persist.tile([P, NT, VW], AV_DT, name="v_sb")
    qT_sb = persist.tile([P, PAIRS, N], ST_DT, name="qT_sb")
    kT_sb = persist.tile([P, PAIRS, N], ST_DT, name="kT_sb")
    oT_sb = persist.tile([P, PAIRS, N], MM_DT, name="oT_sb")
    pw_sb = persist.tile([P, PAIRS, C], MM_DT, name="pw_sb")
    qb_sb = persist.tile([P, PAIRS], F32, name="qb_sb")
    kb_sb = persist.tile([P, PAIRS], F32, name="kb_sb")
    vb_sb = persist.tile([1, VW], MM_DT, name="vb_sb")
    pb_sb = persist.tile([1, C], MM_DT, name="pb_sb")
    # fp32 ones vector (memset can't encode float32r); bitcast where an
    # MM_DT-typed operand is required — the bit pattern is identical.
    ones32 = persist.tile([1, P], F32, name="ones32")

    nc.vector.memset(ones32[:], 1.0)
    # zero via an F32 view: memset can't encode float32r, but 0.0 is all-zero
    # bits in every dtype
    _oT_z = oT_sb[:] if MM_DT != mybir.dt.float32r else oT_sb[:].bitcast(F32)
    nc.vector.memset(_oT_z, 0.0)
    if MM_DT == mybir.dt.bfloat16:
        ones_mm = persist.tile([1, P], MM_DT, name="ones_mm")
        nc.vector.memset(ones_mm[:], 1.0)
    elif MM_DT == mybir.dt.float32r:
        ones_mm = ones32.bitcast(MM_DT)
    else:
        ones_mm = ones32

    # SBUF inputs stay resident for the whole kernel: leftover q^T chunks are
    # computed as PE filler work woven into the attention span.
    phA = ctx.enter_context(tc.tile_pool(name="phA", bufs=1))
    fin_pool = ctx.enter_context(tc.tile_pool(name="fin", bufs=3))
    xT_sb = phA.tile([P, KTC, N], MM_DT, name="xT_sb")
    wq_sb = phA.tile([P, KTC, PAIRS * P], MM_DT, name="wq_sb")
    wk_sb = phA.tile([P, KTC, PAIRS * P], MM_DT, name="wk_sb")
    wv_sb = phA.tile([P, KTC, VW], MM_DT, name="wv_sb")
    # DMAs issued in first-consumption order (the sync engine issues them
    # serially at ~0.65us each, so order = arrival order): k weights and the
    # k^T biases first, then xT column-chunks, then q/v weights (consumed
    # ~15us into the head), and the projection weights (needed last) at the
    # very end — the strided pwP gather is the most expensive descriptor set.
    for kt in range(KTC):
        nc.sync.dma_start(
            wk_sb[:, kt, :], d["wkP"][kt * P : (kt + 1) * P, :]
        )
    nc.sync.dma_start(qb_sb[:], d["qbP"])
    nc.sync.dma_start(kb_sb[:], d["kbP"])
    for ji, (q0c, qwc) in enumerate(CHUNKS):
        for kt in range(KTC):
            nc.sync.dma_start(
                xT_sb[:, kt, q0c : q0c + qwc],
                d["xT"][kt * P : (kt + 1) * P, q0c : q0c + qwc],
            )
        if ji == 0:
            for kt in range(KTC):
                nc.sync.dma_start(
                    wq_sb[:, kt, :], d["wqP"][kt * P : (kt + 1) * P, :]
                )
        if ji == 1:
            for kt in range(KTC):
                nc.sync.dma_start(
                    wv_sb[:, kt, :], d["wvA"][kt * P : (kt + 1) * P, :]
                )
            nc.sync.dma_start(vb_sb[:], d["vbA"])
    nc.sync.dma_start(pw_sb[:], d["pwP"].rearrange("r p m -> p r m"))
    nc.sync.dma_start(pb_sb[:], d["pbR"])

    def emit_qk(pr, q0, qw, pool, tag):
        """q^T and k^T for one (pair, column chunk): pair layout, bias add."""
        psq = pool.tile([P, 512], F32, name="psq", tag=tag)
        psk = pool.tile([P, 512], F32, name="psk", tag=tag)
        for kt in range(KTC):
            nc.tensor.matmul(
                psq[:, 0:qw],
                lhsT=wq_sb[:, kt, pr * P : (pr + 1) * P],
                rhs=xT_sb[:, kt, q0 : q0 + qw],
                start=(kt == 0),
                stop=(kt == KTC - 1),
            )
            nc.tensor.matmul(
                psk[:, 0:qw],
                lhsT=wk_sb[:, kt, pr * P : (pr + 1) * P],
                rhs=xT_sb[:, kt, q0 : q0 + qw],
                start=(kt == 0),
                stop=(kt == KTC - 1),
            )
        nc.vector.tensor_scalar_add(
            qT_sb[:, pr, q0 : q0 + qw], psq[:, 0:qw], qb_sb[:, pr : pr + 1]
        )
        nc.vector.tensor_scalar_add(
            kT_sb[:, pr, q0 : q0 + qw], psk[:, 0:qw], kb_sb[:, pr : pr + 1]
        )

    def emit_k(pr, q0, qw, pool, tag):
        """k^T only (the head computes all of k first)."""
        psk = pool.tile([P, 512], F32, name="psk", tag=tag)
        for kt in range(KTC):
            nc.tensor.matmul(
                psk[:, 0:qw],
                lhsT=wk_sb[:, kt, pr * P : (pr + 1) * P],
                rhs=xT_sb[:, kt, q0 : q0 + qw],
                start=(kt == 0),
                stop=(kt == KTC - 1),
            )
        nc.vector.tensor_scalar_add(
            kT_sb[:, pr, q0 : q0 + qw], psk[:, 0:qw], kb_sb[:, pr : pr + 1]
        )

    def emit_q(pr, q0, qw, pool, tag):
        psq = pool.tile([P, 512], F32, name="psq", tag=tag)
        for kt in range(KTC):
            nc.tensor.matmul(
                psq[:, 0:qw],
                lhsT=wq_sb[:, kt, pr * P : (pr + 1) * P],
                rhs=xT_sb[:, kt, q0 : q0 + qw],
                start=(kt == 0),
                stop=(kt == KTC - 1),
            )
        nc.vector.tensor_scalar_add(
            qT_sb[:, pr, q0 : q0 + qw], psq[:, 0:qw], qb_sb[:, pr : pr + 1]
        )

    def emit_v(nt, pool, tag):
        """v natural [token, 8*(hd|1)]: K=C matmul + rank-1 (ones x vb_aug)
        which adds the v bias AND writes 1.0 into each head's 49th column."""
        psv = pool.tile([P, 512], F32, name="psv", tag=tag)
        for kt in range(KTC):
            nc.tensor.matmul(
                psv[:, 0:VW],
                lhsT=xT_sb[:, kt, nt * P : (nt + 1) * P],
                rhs=wv_sb[:, kt, :],
                start=(kt == 0),
                stop=False,
            )
        nc.tensor.matmul(
            psv[:, 0:VW], lhsT=ones_mm[:, 0:P], rhs=vb_sb[:], start=False, stop=True
        )
        nc.vector.tensor_copy(v_sb[:, nt, :], psv[:, 0:VW])

    def emit_proj(nt, pool, tag):
        """output projection for one token tile; K=113 spans both heads of a
        pair (pw rows 0, 49-63, 64 are zero; oT_sb rows 49-63 zeroed once)."""
        fF = pool.tile([P, 512], F32, name="fF", tag=tag)
        for pr in range(PAIRS):
            nc.tensor.matmul(
                fF[:, 0:C],
                lhsT=oT_sb[0:113, pr, nt * P : (nt + 1) * P],
                rhs=pw_sb[0:113, pr, :],
                start=(pr == 0),
                stop=False,
            )
        nc.tensor.matmul(
            fF[:, 0:C], lhsT=ones_mm[:, 0:P], rhs=pb_sb[:], start=False, stop=True
        )
        ft = fin_pool.tile([P, C], F32, name="ft", tag="ft")
        nc.vector.tensor_copy(ft[:], fF[:, 0:C])
        nc.sync.dma_start(d["out"][nt * P : (nt + 1) * P, :], ft[:])

    # ---------------- attention: flash, chunk-major, S^T layout ----------
    # Chunk-major (q-chunk outer, pairs inner) so each chunk's output
    # projection can run as PE filler right after its 4th pair completes,
    # instead of serializing 50us of projection after all attention.
    AV_LAG = 6  # attn@V trails exp by this many groups in steady state
    es_pool = ctx.enter_context(tc.tile_pool(name="es", bufs=8))
    rc_pool = ctx.enter_context(tc.tile_pool(name="rcp", bufs=4))
    psS = ctx.enter_context(tc.tile_pool(name="psS", bufs=1, space="PSUM"))
    # alternating 3-slot/2-slot score groups: two tags of one buf each act
    # as a double buffer in 5 banks
    GSIZES = [3, 2] * 6 + [3, 3]
    seq = [(kt, hoff) for kt in range(NT) for hoff in (0, 64)]
    # psO/psX open only after the head's PSUM pool closes (8-bank budget);
    # attn@V accumulators are therefore allocated lazily at first drain.
    psO_ref = [None]
    psX_ref = [None]
    av_q = []

    # filler queues: leftover q^T chunks (needed just-in-time) and completed
    # chunks' projections, emitted 2 per pair-block through the single spare
    # PSUM bank. Keeps the PE dense (HAM stays un-throttled) and moves the
    # projection under the ACT-bound attention span.
    qk_fill = [(c, pr) for c in range(1, len(CHUNKS)) for pr in range(PAIRS)]
    proj_fill = []

    def pop_filler(ci):
        if qk_fill and qk_fill[0][0] <= ci + 1:
            c, pr = qk_fill.pop(0)
            emit_q(pr, CHUNKS[c][0], CHUNKS[c][1], psX_ref[0], "fx")
        elif proj_fill:
            emit_proj(proj_fill.pop(0), psX_ref[0], "fx")
        elif qk_fill:
            c, pr = qk_fill.pop(0)
            emit_q(pr, CHUNKS[c][0], CHUNKS[c][1], psX_ref[0], "fx")

    def attnv(est, si, gs, blk):
        if blk["oTP"] is None:
            blk["oTP"] = psO_ref[0].tile([P, 512], F32, name="oTP", tag="oTP")
        for j in range(gs):
            kt2, hoff2 = seq[si + j]
            h = blk["pr"] * 2 + (0 if hoff2 == 0 else 1)
            nc.tensor.matmul(
                blk["oTP"][hoff2 : hoff2 + HD + 1, 0 : blk["qw"]],
                lhsT=v_sb[:, kt2, h * (HD + 1) : (h + 1) * (HD + 1)],
                rhs=est[:, j, 0 : blk["qw"]],
                start=(kt2 == 0),
                stop=(kt2 == NT - 1),
            )
        if si + gs == 2 * NT:
            normalize(blk)

    def emit_stq_group(blk, gi, gs, si):
        """One score group: S^T matmuls into PSUM, exp to SBUF, queue attn@V."""
        pr, q0, qw = blk["pr"], blk["q0"], blk["qw"]
        sg = psS.tile([P, gs, 512], F32, name="sg", tag=f"sg{gs}")
        for j in range(gs):
            kt, hoff = seq[si + j]
            nc.tensor.matmul(
                sg[:, j, 0:qw],
                lhsT=kT_sb[hoff : hoff + HD, pr, kt * P : (kt + 1) * P],
                rhs=qT_sb[hoff : hoff + HD, pr, q0 : q0 + qw],
                start=True,
                stop=True,
            )
        est = es_pool.tile([P, gs, 512], AV_DT, name="est", tag=f"est{gs}")
        nc.scalar.activation(est[:, :, 0:qw], sg[:, :, 0:qw], _EXP, scale=SCALE)
        av_q.append((est, si, gs, blk))

    def normalize(blk):
        # ~51-ULP approx reciprocals straight off the two PSUM
        # denominator rows (0 / 64), both landing at a base-0 SBUF row
        # (cross-base is legal for single-input ops when both bases are
        # 32-aligned). GPSIMD (otherwise idle) broadcasts them across
        # partitions: recA over rows 0-48, recB over rows 0-112 so the
        # head-B multiply can read the 64:113 slice with matching input
        # bases. The normalize is two DVE multiplies reading the
        # accumulator PSUM directly — no PE work, no drain copies.
        oTP, pr, q0, qw = blk["oTP"], blk["pr"], blk["q0"], blk["qw"]
        recA = rc_pool.tile([1, 512], F32, name="recA", tag="rcA")
        recB = rc_pool.tile([1, 512], F32, name="recB", tag="rcB")
        nc.vector.reciprocal_approx_fast(recA[0:1, 0:qw], oTP[0:1, 0:qw])
        nc.vector.reciprocal_approx_fast(recB[0:1, 0:qw], oTP[64:65, 0:qw])
        bcsA = rc_pool.tile([P, 512], F32, name="bcsA", tag="bcsA")
        bcsB = rc_pool.tile([P, 512], F32, name="bcsB", tag="bcsB")
        nc.gpsimd.partition_broadcast(bcsA[0 : HD + 1, 0:qw], recA[0:1, 0:qw])
        nc.gpsimd.partition_broadcast(
            bcsB[0 : 64 + HD + 1, 0:qw], recB[0:1, 0:qw]
        )
        nc.vector.tensor_mul(
            oT_sb[0 : HD + 1, pr, q0 : q0 + qw],
            oTP[0 : HD + 1, 0:qw],
            bcsA[0 : HD + 1, 0:qw],
        )
        nc.vector.tensor_mul(
            oT_sb[64 : 64 + HD + 1, pr, q0 : q0 + qw],
            oTP[64 : 64 + HD + 1, 0:qw],
            bcsB[64 : 64 + HD + 1, 0:qw],
        )
        # once a chunk's 4th pair is normalized, its token tiles are
        # fully attended — queue their output projections as filler
        if pr == PAIRS - 1:
            for nt in range(q0 // P, (q0 + qw) // P):
                proj_fill.append(nt)

    # ---------------- head, overlapped with the first attention block -----
    # Only pair 0's k^T and chunk-0 q^T gate the first scores. The rest of
    # the head (k^T for pairs 1-3, chunk-0 q^T, all of v) interleaves with
    # block (c0,p0)'s S^T+exp groups, 3 units per group, through a 3-bank
    # PSUM pool (3 + sg's 5 = 8). attn@V for block 0 is fully deferred —
    # its accumulator pool can only open once the head pool closes — and
    # catches up during block (c0,p1), at most 2 drains per group.
    with tc.tile_pool(name="psH", bufs=3, space="PSUM") as psH:
        # critical path to the first scores: just k^T/q^T of (pair 0, chunk
        # 0) — block 0's S^T groups consume key tiles in ascending order, so
        # the remaining k^T column chunks arrive as interleaved head units
        # well before the groups that read them.
        emit_k(0, CHUNKS[0][0], CHUNKS[0][1], psH, "ph")
        emit_q(0, CHUNKS[0][0], CHUNKS[0][1], psH, "ph")

        head_units = []
        for q0h, qwh in CHUNKS[1:]:
            head_units.append(
                lambda q0h=q0h, qwh=qwh: emit_k(0, q0h, qwh, psH, "ph")
            )
        for pr in range(1, PAIRS):
            for q0h, qwh in CHUNKS:
                head_units.append(
                    lambda pr=pr, q0h=q0h, qwh=qwh: emit_k(pr, q0h, qwh, psH, "ph")
                )
            head_units.append(
                lambda pr=pr: emit_q(pr, CHUNKS[0][0], CHUNKS[0][1], psH, "ph")
            )
        for nt in range(NT):
            head_units.append(lambda nt=nt: emit_v(nt, psH, "ph"))

        blk0 = {"oTP": None, "pr": 0, "q0": CHUNKS[0][0], "qw": CHUNKS[0][1]}
        si = 0
        for gi, gs in enumerate(GSIZES):
            for _ in range(3):
                if head_units:
                    head_units.pop(0)()
            emit_stq_group(blk0, gi, gs, si)
            si += gs
        while head_units:
            head_units.pop(0)()

    with (
        tc.tile_pool(name="psO", bufs=2, space="PSUM") as psO,
        tc.tile_pool(name="psX", bufs=1, space="PSUM") as psX,
    ):
        psO_ref[0] = psO
        psX_ref[0] = psX
        # The attn@V queue carries ACROSS pair-blocks: a block's tail
        # attn@V matmuls interleave with the next block's first S^T groups
        # instead of flushing serially at the boundary (which parked the
        # in-order PE queue and starved ACT for ~2.5us per block).
        for cidx, (q0, qw) in enumerate(CHUNKS):
            # correctness: this chunk's q^T must be emitted before any S^T
            # references it
            while qk_fill and qk_fill[0][0] <= cidx:
                c, pr = qk_fill.pop(0)
                emit_q(pr, CHUNKS[c][0], CHUNKS[c][1], psX, "fx")
            for pr in range(PAIRS):
                if cidx == 0 and pr == 0:
                    continue  # emitted inside the head overlap above
                # one accumulator bank for both heads: head A at partitions
                # 0-48 (col strips 0-1), head B at 64-112 (strips 2-3) — the
                # two matmuls stay concurrent via col tiling, and bufs=2
                # double-buffers the bank across blocks so this block's
                # attn@V never waits on the previous block's normalize.
                blk = {"oTP": None, "pr": pr, "q0": q0, "qw": qw}
                si = 0
                # more filler slots in the last chunks so the remaining
                # projections drain before the attention stream ends
                slots = (4, 10) if cidx < 3 else (2, 6, 10)
                for gi, gs in enumerate(GSIZES):
                    if gi in slots:
                        pop_filler(cidx)
                    emit_stq_group(blk, gi, gs, si)
                    pops = 0
                    while len(av_q) > AV_LAG and pops < 2:
                        attnv(*av_q.pop(0))
                        pops += 1
                    si += gs

        # drain the attn@V tail, then whatever filler work remains
        for av in av_q:
            attnv(*av)
        av_q.clear()
        while qk_fill or proj_fill:
            pop_filler(len(CHUNKS))


def build_program(n_cores: int = 8):
    nc = bacc.Bacc(
        "TRN2",
        target_bir_lowering=False,
        debug=False,
        enable_asserts=False,
        num_devices=n_cores,
    )
    d = {
        "xT": nc.dram_tensor("xT", [C, N], MM_DT, kind="ExternalInput").ap(),
        "wqP": nc.dram_tensor("wqP", [C, PAIRS * P], MM_DT, kind="ExternalInput").ap(),
        "wkP": nc.dram_tensor("wkP", [C, PAIRS * P], MM_DT, kind="ExternalInput").ap(),
        "wvA": nc.dram_tensor("wvA", [C, VW], MM_DT, kind="ExternalInput").ap(),
        "vbA": nc.dram_tensor("vbA", [1, VW], MM_DT, kind="ExternalInput").ap(),
        "qbP": nc.dram_tensor("qbP", [P, PAIRS], F32, kind="ExternalInput").ap(),
        "kbP": nc.dram_tensor("kbP", [P, PAIRS], F32, kind="ExternalInput").ap(),
        "pwP": nc.dram_tensor("pwP", [PAIRS, P, C], MM_DT, kind="ExternalInput").ap(),
        "pbR": nc.dram_tensor("pbR", [1, C], MM_DT, kind="ExternalInput").ap(),
        "out": nc.dram_tensor("out", [N, C], F32, kind="ExternalOutput").ap(),
    }
    import contextlib

    with tile.TileContext(nc) as tc:
        with contextlib.ExitStack() as ctx:
            _emit(tc, d, ctx)
    nc.finalize()
    return nc


def _mm_np_dtype():
    if MM_DT == mybir.dt.bfloat16:
        import ml_dtypes

        return ml_dtypes.bfloat16
    return np.float32


def _prep_host(x, q_w, q_b, kv_w, kv_b, proj_w, proj_b):
    """Transpose/pack on host. Returns (per-core xT list, shared map)."""
    f32 = np.float32
    x = np.asarray(x, f32)
    xT = np.ascontiguousarray(x.reshape(B, N, C).transpose(0, 2, 1))  # [B, C, N]

    qwT = np.ascontiguousarray(np.asarray(q_w, f32).T)  # [Cin, Cout]
    kwT = np.ascontiguousarray(np.asarray(kv_w[:C], f32).T)
    vwT = np.ascontiguousarray(np.asarray(kv_w[C:], f32).T)
    pwT = np.ascontiguousarray(np.asarray(proj_w, f32).T)

    wqP = np.zeros((C, PAIRS * P), f32)
    wkP = np.zeros((C, PAIRS * P), f32)
    qbP = np.zeros((P, PAIRS), f32)
    kbP = np.zeros((P, PAIRS), f32)
    pwP = np.zeros((PAIRS, P, C), f32)
    for p in range(PAIRS):
        a, b = 2 * p, 2 * p + 1
        wqP[:, p * P : p * P + HD] = qwT[:, a * HD : (a + 1) * HD]
        wqP[:, p * P + 64 : p * P + 64 + HD] = qwT[:, b * HD : (b + 1) * HD]
        wkP[:, p * P : p * P + HD] = kwT[:, a * HD : (a + 1) * HD]
        wkP[:, p * P + 64 : p * P + 64 + HD] = kwT[:, b * HD : (b + 1) * HD]
        qbP[0:HD, p] = q_b[a * HD : (a + 1) * HD]
        qbP[64 : 64 + HD, p] = q_b[b * HD : (b + 1) * HD]
        kbP[0:HD, p] = kv_b[a * HD : (a + 1) * HD]
        kbP[64 : 64 + HD, p] = kv_b[b * HD : (b + 1) * HD]
        # rows 1..48 / 65..112 carry the proj weights; rows 0 / 64 stay zero
        # to swallow the denominator row of outT.
        pwP[p, 1 : 1 + HD, :] = pwT[a * HD : (a + 1) * HD, :]
        pwP[p, 65 : 65 + HD, :] = pwT[b * HD : (b + 1) * HD, :]

    # V blocks are [ones | v0..v47] per head so the softmax denominator lands
    # at a 32-aligned PSUM partition (0 / 64).
    wvA = np.zeros((C, VW), f32)
    vbA = np.zeros((1, VW), f32)
    for h in range(NH):
        wvA[:, h * (HD + 1) + 1 : (h + 1) * (HD + 1)] = vwT[:, h * HD : (h + 1) * HD]
        vbA[0, h * (HD + 1) + 1 : (h + 1) * (HD + 1)] = kv_b[
            C + h * HD : C + (h + 1) * HD
        ]
        vbA[0, h * (HD + 1)] = 1.0

    mmdt = _mm_np_dtype()
    shared = {
        "wqP": wqP.astype(mmdt),
        "wkP": wkP.astype(mmdt),
        "wvA": wvA.astype(mmdt),
        "vbA": vbA.astype(mmdt),
        "qbP": qbP,
        "kbP": kbP,
        "pwP": pwP.astype(mmdt),
        "pbR": np.asarray(proj_b, f32).reshape(1, C).astype(mmdt),
    }
    return xT.astype(mmdt), shared


_PROGRAM = None


def _get_program():
    global _PROGRAM
    if _PROGRAM is None:
        _PROGRAM = build_program(B)
    return _PROGRAM


def kernel(x, q_w, q_b, kv_w, kv_b, proj_w, proj_b):
    xT, shared = _prep_host(x, q_w, q_b, kv_w, kv_b, proj_w, proj_b)
    nc = _get_program()
    in_maps = [dict(shared, xT=np.ascontiguousarray(xT[b])) for b in range(B)]
    res = run_bass_kernel_spmd(nc, in_maps, list(range(B)))
    outs = [np.asarray(res.results[i]["out"], np.float32) for i in range(B)]
    return np.stack(outs).reshape(B, HH, WW, C)



# revision 32
# speedup vs baseline: 1.0045x; 1.0045x over previous
"""Trainium2 Bass kernel for nn_Attention_17532056502607.

Multi-head self-attention (B=8, N=48*48=2304 tokens, C=384, 8 heads of 48):
    q = x @ q_w.T + q_b ; k,v = x @ kv_w.T + kv_b
    out = softmax(q k^T / sqrt(48)) v ; y = out @ proj_w.T + proj_b

Sharding: data-parallel, one batch element per NeuronCore (8 cores).

Per-core algorithm (all in "S^T layout", keys on partitions — no transposes):
  - host supplies xT = x_b^T [C, N] and head-PAIR-packed weights: heads 2p /
    2p+1 of a pair live at partition rows 0-47 / 64-111, so two K=48 matmuls
    run concurrently in the PE array (row/col 32-tiles).
  - qT/kT [C_pair, N] = wT-pair @ xT          (PE, K=C=384)
  - v    [N, 8*49]    = x @ wv + rank-1 bias matmul; each head's V block is
    [ones | v0..v47], so attn@V also accumulates the softmax denominator at
    a 32-aligned output partition (0 / 64).
  - S^T  [keys, q]    = kT-tile.T @ qT        (K=48, row-packed head pairs)
  - expS = exp(scale * S^T)                   (ACT, reads PSUM, writes SBUF)
  - outT [49x2, q]   += (1|v).T @ expS        (K=128 keys, col strips 0/64)
  - normalize: drain to SBUF, exact reciprocal of rows 0/64, rank-1 selector
    matmul broadcasts the recips, DVE multiply.
  - y    [N, C]       = sum_pairs outT-pair.T @ projw-pair + bias, with K=113
    spanning both head blocks and zero weight rows under the denominators.

Matmul dtypes default to float32r for x->q/k/v and the output projection and
bf16 for the attention core (rel err ~2.7e-3 vs the fp32 reference; set
ATTN_MM_DT=float32 for exact-but-slow).
"""

import os
import sys

import numpy as np

for _p in ("/opt/trn_rl_repo",):
    if _p not in sys.path:
        sys.path.append(_p)

import concourse.bass as bass  # noqa: E402
import concourse.tile as tile  # noqa: E402
from concourse import bacc, mybir  # noqa: E402
from concourse.bass_utils import run_bass_kernel_spmd  # noqa: E402

# ---------------------------------------------------------------- constants
B = 8
HH = 48
WW = 48
C = 384
N = HH * WW  # 2304
NH = 8
HD = 48
PAIRS = NH // 2  # 4
P = 128
NT = N // P  # 18 token tiles
KTC = C // P  # 3 contraction tiles over C
SCALE = float(HD) ** -0.5
VW = NH * (HD + 1)  # 392: v with a ones column per head
CHUNKS = [(0, 512), (512, 512), (1024, 512), (1536, 512), (2048, 256)]

F32 = mybir.dt.float32
# Matmul dtype for all SBUF operands. float32 = exact but 4 cyc/row on PE;
# float32r = same bits, reduced-precision single-pass matmul (1 cyc/row for
# moving dim >= 256) but cannot write PSUM at partition base 64; bfloat16
# halves SBUF/DMA and enables FWL.
MM_DT = getattr(mybir.dt, os.environ.get("ATTN_MM_DT", "bfloat16"))
# attn@V runs in bf16 when MM_DT is float32r (f32r matmuls cannot col-tile to
# partition base 64; bf16 error here is averaged over the 2304-key softmax).
AV_DT = (
    mybir.dt.bfloat16
    if MM_DT == mybir.dt.float32r
    else getattr(mybir.dt, os.environ.get("ATTN_AV_DT", MM_DT.value))
)

# S^T (q@k) operand dtype. bf16 emits separate LDWEIGHTS instructions that
# overlap prior matmuls in the other row group (fp32r self-loads weights
# serially); the softmax averages away the extra rounding (+6% rel err).
ST_DT = getattr(
    mybir.dt,
    os.environ.get(
        "ATTN_ST_DT",
        "bfloat16" if MM_DT == mybir.dt.float32r else MM_DT.value,
    ),
)

# broadcast-matmul operand dtype: f32r is 4x faster on PE and legal at dst
# base 0; producers must write f32r-typed outputs (verifier checks rounding)
BC_DT = mybir.dt.float32r if MM_DT != mybir.dt.float32 else F32

_EXP = mybir.ActivationFunctionType.Exp


def _emit(tc: tile.TileContext, d: dict, ctx):
    nc = tc.nc

    persist = ctx.enter_context(tc.tile_pool(name="persist", bufs=1))
    v_sb = persist.tile([P, NT, VW], AV_DT, name="v_sb")
    qT_sb = persist.tile([P, PAIRS, N], ST_DT, name="qT_sb")
    kT_sb = persist.tile([P, PAIRS, N], ST_DT, name="kT_sb")
    oT_sb = persist.tile([P, PAIRS, N], MM_DT, name="oT_sb")
    pw_sb = persist.tile([P, PAIRS, C], MM_DT, name="pw_sb")
    qb_sb = persist.tile([P, PAIRS], F32, name="qb_sb")
    kb_sb = persist.tile([P, PAIRS], F32, name="kb_sb")
    vb_sb = persist.tile([1, VW], MM_DT, name="vb_sb")
    pb_sb = persist.tile([1, C], MM_DT, name="pb_sb")
    # fp32 ones vector (memset can't encode float32r); bitcast where an
    # MM_DT-typed operand is required — the bit pattern is identical.
    ones32 = persist.tile([1, P], F32, name="ones32")

    nc.vector.memset(ones32[:], 1.0)
    # zero via an F32 view: memset can't encode float32r, but 0.0 is all-zero
    # bits in every dtype
    _oT_z = oT_sb[:] if MM_DT != mybir.dt.float32r else oT_sb[:].bitcast(F32)
    nc.vector.memset(_oT_z, 0.0)
    if MM_DT == mybir.dt.bfloat16:
        ones_mm = persist.tile([1, P], MM_DT, name="ones_mm")
        nc.vector.memset(ones_mm[:], 1.0)
    elif MM_DT == mybir.dt.float32r:
        ones_mm = ones32.bitcast(MM_DT)
    else:
        ones_mm = ones32

    # SBUF inputs stay resident for the whole kernel: leftover q^T chunks are
    # computed as PE filler work woven into the attention span.
    phA = ctx.enter_context(tc.tile_pool(name="phA", bufs=1))
    fin_pool = ctx.enter_context(tc.tile_pool(name="fin", bufs=3))
    xT_sb = phA.tile([P, KTC, N], MM_DT, name="xT_sb")
    wq_sb = phA.tile([P, KTC, PAIRS * P], MM_DT, name="wq_sb")
    wk_sb = phA.tile([P, KTC, PAIRS * P], MM_DT, name="wk_sb")
    wv_sb = phA.tile([P, KTC, VW], MM_DT, name="wv_sb")
    # DMAs issued in first-consumption order (the sync engine issues them
    # serially at ~0.65us each, so order = arrival order): k weights and the
    # k^T biases first, then xT column-chunks, then q/v weights (consumed
    # ~15us into the head), and the projection weights (needed last) at the
    # very end — the strided pwP gather is the most expensive descriptor set.
    for kt in range(KTC):
        nc.sync.dma_start(
            wk_sb[:, kt, :], d["wkP"][kt * P : (kt + 1) * P, :]
        )
    nc.sync.dma_start(qb_sb[:], d["qbP"])
    nc.sync.dma_start(kb_sb[:], d["kbP"])
    for ji, (q0c, qwc) in enumerate(CHUNKS):
        for kt in range(KTC):
            nc.sync.dma_start(
                xT_sb[:, kt, q0c : q0c + qwc],
                d["xT"][kt * P : (kt + 1) * P, q0c : q0c + qwc],
            )
        if ji == 0:
            for kt in range(KTC):
                nc.sync.dma_start(
                    wq_sb[:, kt, :], d["wqP"][kt * P : (kt + 1) * P, :]
                )
        if ji == 1:
            for kt in range(KTC):
                nc.sync.dma_start(
                    wv_sb[:, kt, :], d["wvA"][kt * P : (kt + 1) * P, :]
                )
            nc.sync.dma_start(vb_sb[:], d["vbA"])
    nc.sync.dma_start(pw_sb[:], d["pwP"].rearrange("r p m -> p r m"))
    nc.sync.dma_start(pb_sb[:], d["pbR"])

    def emit_qk(pr, q0, qw, pool, tag):
        """q^T and k^T for one (pair, column chunk): pair layout, bias add."""
        psq = pool.tile([P, 512], F32, name="psq", tag=tag)
        psk = pool.tile([P, 512], F32, name="psk", tag=tag)
        for kt in range(KTC):
            nc.tensor.matmul(
                psq[:, 0:qw],
                lhsT=wq_sb[:, kt, pr * P : (pr + 1) * P],
                rhs=xT_sb[:, kt, q0 : q0 + qw],
                start=(kt == 0),
                stop=(kt == KTC - 1),
            )
            nc.tensor.matmul(
                psk[:, 0:qw],
                lhsT=wk_sb[:, kt, pr * P : (pr + 1) * P],
                rhs=xT_sb[:, kt, q0 : q0 + qw],
                start=(kt == 0),
                stop=(kt == KTC - 1),
            )
        nc.vector.tensor_scalar_add(
            qT_sb[:, pr, q0 : q0 + qw], psq[:, 0:qw], qb_sb[:, pr : pr + 1]
        )
        nc.vector.tensor_scalar_add(
            kT_sb[:, pr, q0 : q0 + qw], psk[:, 0:qw], kb_sb[:, pr : pr + 1]
        )

    def emit_k(pr, q0, qw, pool, tag):
        """k^T only (the head computes all of k first)."""
        psk = pool.tile([P, 512], F32, name="psk", tag=tag)
        for kt in range(KTC):
            nc.tensor.matmul(
                psk[:, 0:qw],
                lhsT=wk_sb[:, kt, pr * P : (pr + 1) * P],
                rhs=xT_sb[:, kt, q0 : q0 + qw],
                start=(kt == 0),
                stop=(kt == KTC - 1),
            )
        nc.vector.tensor_scalar_add(
            kT_sb[:, pr, q0 : q0 + qw], psk[:, 0:qw], kb_sb[:, pr : pr + 1]
        )

    def emit_q(pr, q0, qw, pool, tag):
        psq = pool.tile([P, 512], F32, name="psq", tag=tag)
        for kt in range(KTC):
            nc.tensor.matmul(
                psq[:, 0:qw],
                lhsT=wq_sb[:, kt, pr * P : (pr + 1) * P],
                rhs=xT_sb[:, kt, q0 : q0 + qw],
                start=(kt == 0),
                stop=(kt == KTC - 1),
            )
        nc.vector.tensor_scalar_add(
            qT_sb[:, pr, q0 : q0 + qw], psq[:, 0:qw], qb_sb[:, pr : pr + 1]
        )

    def emit_v(nt, pool, tag):
        """v natural [token, 8*(hd|1)]: K=C matmul + rank-1 (ones x vb_aug)
        which adds the v bias AND writes 1.0 into each head's 49th column."""
        psv = pool.tile([P, 512], F32, name="psv", tag=tag)
        for kt in range(KTC):
            nc.tensor.matmul(
                psv[:, 0:VW],
                lhsT=xT_sb[:, kt, nt * P : (nt + 1) * P],
                rhs=wv_sb[:, kt, :],
                start=(kt == 0),
                stop=False,
            )
        nc.tensor.matmul(
            psv[:, 0:VW], lhsT=ones_mm[:, 0:P], rhs=vb_sb[:], start=False, stop=True
        )
        nc.vector.tensor_copy(v_sb[:, nt, :], psv[:, 0:VW])

    def emit_proj(nt, pool, tag):
        """output projection for one token tile; K=113 spans both heads of a
        pair (pw rows 0, 49-63, 64 are zero; oT_sb rows 49-63 zeroed once)."""
        fF = pool.tile([P, 512], F32, name="fF", tag=tag)
        for pr in range(PAIRS):
            nc.tensor.matmul(
                fF[:, 0:C],
                lhsT=oT_sb[0:113, pr, nt * P : (nt + 1) * P],
                rhs=pw_sb[0:113, pr, :],
                start=(pr == 0),
                stop=False,
            )
        nc.tensor.matmul(
            fF[:, 0:C], lhsT=ones_mm[:, 0:P], rhs=pb_sb[:], start=False, stop=True
        )
        ft = fin_pool.tile([P, C], F32, name="ft", tag="ft")
        nc.vector.tensor_copy(ft[:], fF[:, 0:C])
        nc.sync.dma_start(d["out"][nt * P : (nt + 1) * P, :], ft[:])

    # ---------------- attention: flash, chunk-major, S^T layout ----------
    # Chunk-major (q-chunk outer, pairs inner) so each chunk's output
    # projection can run as PE filler right after its 4th pair completes,
    # instead of serializing 50us of projection after all attention.
    AV_LAG = 4  # attn@V trails exp by this many groups in steady state
    es_pool = ctx.enter_context(tc.tile_pool(name="es", bufs=8))
    rc_pool = ctx.enter_context(tc.tile_pool(name="rcp", bufs=4))
    psS = ctx.enter_context(tc.tile_pool(name="psS", bufs=1, space="PSUM"))
    # alternating 3-slot/2-slot score groups: two tags of one buf each act
    # as a double buffer in 5 banks
    GSIZES = [3, 2] * 6 + [3, 3]
    seq = [(kt, hoff) for kt in range(NT) for hoff in (0, 64)]
    # psO/psX open only after the head's PSUM pool closes (8-bank budget);
    # attn@V accumulators are therefore allocated lazily at first drain.
    psO_ref = [None]
    psX_ref = [None]
    av_q = []

    # filler queues: leftover q^T chunks (needed just-in-time) and completed
    # chunks' projections, emitted 2 per pair-block through the single spare
    # PSUM bank. Keeps the PE dense (HAM stays un-throttled) and moves the
    # projection under the ACT-bound attention span.
    qk_fill = [(c, pr) for c in range(1, len(CHUNKS)) for pr in range(PAIRS)]
    proj_fill = []

    def pop_filler(ci):
        if qk_fill and qk_fill[0][0] <= ci + 1:
            c, pr = qk_fill.pop(0)
            emit_q(pr, CHUNKS[c][0], CHUNKS[c][1], psX_ref[0], "fx")
        elif proj_fill:
            emit_proj(proj_fill.pop(0), psX_ref[0], "fx")
        elif qk_fill:
            c, pr = qk_fill.pop(0)
            emit_q(pr, CHUNKS[c][0], CHUNKS[c][1], psX_ref[0], "fx")

    def attnv(est, si, gs, blk):
        if blk["oTP"] is None:
            blk["oTP"] = psO_ref[0].tile([P, 512], F32, name="oTP", tag="oTP")
        for j in range(gs):
            kt2, hoff2 = seq[si + j]
            h = blk["pr"] * 2 + (0 if hoff2 == 0 else 1)
            nc.tensor.matmul(
                blk["oTP"][hoff2 : hoff2 + HD + 1, 0 : blk["qw"]],
                lhsT=v_sb[:, kt2, h * (HD + 1) : (h + 1) * (HD + 1)],
                rhs=est[:, j, 0 : blk["qw"]],
                start=(kt2 == 0),
                stop=(kt2 == NT - 1),
            )
        if si + gs == 2 * NT:
            normalize(blk)

    def emit_stq_group(blk, gi, gs, si):
        """One score group: S^T matmuls into PSUM, exp to SBUF, queue attn@V."""
        pr, q0, qw = blk["pr"], blk["q0"], blk["qw"]
        sg = psS.tile([P, gs, 512], F32, name="sg", tag=f"sg{gs}")
        for j in range(gs):
            kt, hoff = seq[si + j]
            nc.tensor.matmul(
                sg[:, j, 0:qw],
                lhsT=kT_sb[hoff : hoff + HD, pr, kt * P : (kt + 1) * P],
                rhs=qT_sb[hoff : hoff + HD, pr, q0 : q0 + qw],
                start=True,
                stop=True,
            )
        est = es_pool.tile([P, gs, 512], AV_DT, name="est", tag=f"est{gs}")
        nc.scalar.activation(est[:, :, 0:qw], sg[:, :, 0:qw], _EXP, scale=SCALE)
        av_q.append((est, si, gs, blk))

    def normalize(blk):
        # ~51-ULP approx reciprocals straight off the two PSUM
        # denominator rows (0 / 64), both landing at a base-0 SBUF row
        # (cross-base is legal for single-input ops when both bases are
        # 32-aligned). GPSIMD (otherwise idle) broadcasts them across
        # partitions: recA over rows 0-48, recB over rows 0-112 so the
        # head-B multiply can read the 64:113 slice with matching input
        # bases. The normalize is two DVE multiplies reading the
        # accumulator PSUM directly — no PE work, no drain copies.
        oTP, pr, q0, qw = blk["oTP"], blk["pr"], blk["q0"], blk["qw"]
        recA = rc_pool.tile([1, 512], F32, name="recA", tag="rcA")
        recB = rc_pool.tile([1, 512], F32, name="recB", tag="rcB")
        nc.vector.reciprocal_approx_fast(recA[0:1, 0:qw], oTP[0:1, 0:qw])
        nc.vector.reciprocal_approx_fast(recB[0:1, 0:qw], oTP[64:65, 0:qw])
        bcsA = rc_pool.tile([P, 512], F32, name="bcsA", tag="bcsA")
        bcsB = rc_pool.tile([P, 512], F32, name="bcsB", tag="bcsB")
        nc.gpsimd.partition_broadcast(bcsA[0 : HD + 1, 0:qw], recA[0:1, 0:qw])
        nc.gpsimd.partition_broadcast(
            bcsB[0 : 64 + HD + 1, 0:qw], recB[0:1, 0:qw]
        )
        nc.vector.tensor_mul(
            oT_sb[0 : HD + 1, pr, q0 : q0 + qw],
            oTP[0 : HD + 1, 0:qw],
            bcsA[0 : HD + 1, 0:qw],
        )
        nc.vector.tensor_mul(
            oT_sb[64 : 64 + HD + 1, pr, q0 : q0 + qw],
            oTP[64 : 64 + HD + 1, 0:qw],
            bcsB[64 : 64 + HD + 1, 0:qw],
        )
        # once a chunk's 4th pair is normalized, its token tiles are
        # fully attended — queue their output projections as filler
        if pr == PAIRS - 1:
            for nt in range(q0 // P, (q0 + qw) // P):
                proj_fill.append(nt)

    # ---------------- head, overlapped with the first attention block -----
    # Only pair 0's k^T and chunk-0 q^T gate the first scores. The rest of
    # the head (k^T for pairs 1-3, chunk-0 q^T, all of v) interleaves with
    # block (c0,p0)'s S^T+exp groups, 3 units per group, through a 3-bank
    # PSUM pool (3 + sg's 5 = 8). attn@V for block 0 is fully deferred —
    # its accumulator pool can only open once the head pool closes — and
    # catches up during block (c0,p1), at most 2 drains per group.
    with tc.tile_pool(name="psH", bufs=3, space="PSUM") as psH:
        # critical path to the first scores: just k^T/q^T of (pair 0, chunk
        # 0) — block 0's S^T groups consume key tiles in ascending order, so
        # the remaining k^T column chunks arrive as interleaved head units
        # well before the groups that read them.
        emit_k(0, CHUNKS[0][0], CHUNKS[0][1], psH, "ph")
        emit_q(0, CHUNKS[0][0], CHUNKS[0][1], psH, "ph")

        head_units = []
        for q0h, qwh in CHUNKS[1:]:
            head_units.append(
                lambda q0h=q0h, qwh=qwh: emit_k(0, q0h, qwh, psH, "ph")
            )
        for pr in range(1, PAIRS):
            for q0h, qwh in CHUNKS:
                head_units.append(
                    lambda pr=pr, q0h=q0h, qwh=qwh: emit_k(pr, q0h, qwh, psH, "ph")
                )
            head_units.append(
                lambda pr=pr: emit_q(pr, CHUNKS[0][0], CHUNKS[0][1], psH, "ph")
            )
        for nt in range(NT):
            head_units.append(lambda nt=nt: emit_v(nt, psH, "ph"))

        blk0 = {"oTP": None, "pr": 0, "q0": CHUNKS[0][0], "qw": CHUNKS[0][1]}
        si = 0
        for gi, gs in enumerate(GSIZES):
            # S^T first: the popped head units may still be waiting on their
            # xT DMA chunks, and the in-order PE queue would park the first
            # scores behind them
            emit_stq_group(blk0, gi, gs, si)
            for _ in range(3):
                if head_units:
                    head_units.pop(0)()
            si += gs
        while head_units:
            head_units.pop(0)()

    with (
        tc.tile_pool(name="psO", bufs=2, space="PSUM") as psO,
        tc.tile_pool(name="psX", bufs=1, space="PSUM") as psX,
    ):
        psO_ref[0] = psO
        psX_ref[0] = psX
        # The attn@V queue carries ACROSS pair-blocks: a block's tail
        # attn@V matmuls interleave with the next block's first S^T groups
        # instead of flushing serially at the boundary (which parked the
        # in-order PE queue and starved ACT for ~2.5us per block).
        for cidx, (q0, qw) in enumerate(CHUNKS):
            # correctness: this chunk's q^T must be emitted before any S^T
            # references it
            while qk_fill and qk_fill[0][0] <= cidx:
                c, pr = qk_fill.pop(0)
                emit_q(pr, CHUNKS[c][0], CHUNKS[c][1], psX, "fx")
            for pr in range(PAIRS):
                if cidx == 0 and pr == 0:
                    continue  # emitted inside the head overlap above
                # one accumulator bank for both heads: head A at partitions
                # 0-48 (col strips 0-1), head B at 64-112 (strips 2-3) — the
                # two matmuls stay concurrent via col tiling, and bufs=2
                # double-buffers the bank across blocks so this block's
                # attn@V never waits on the previous block's normalize.
                blk = {"oTP": None, "pr": pr, "q0": q0, "qw": qw}
                si = 0
                # more filler slots in the last chunks so the remaining
                # projections drain before the attention stream ends
                slots = (4, 10) if cidx < 3 else (2, 6, 10)
                for gi, gs in enumerate(GSIZES):
                    if gi in slots:
                        pop_filler(cidx)
                    emit_stq_group(blk, gi, gs, si)
                    pops = 0
                    while len(av_q) > AV_LAG and pops < 2:
                        attnv(*av_q.pop(0))
                        pops += 1
                    si += gs

        # drain the attn@V tail, then whatever filler work remains
        for av in av_q:
            attnv(*av)
        av_q.clear()
        while qk_fill or proj_fill:
            pop_filler(len(CHUNKS))


def build_program(n_cores: int = 8):
    nc = bacc.Bacc(
        "TRN2",
        target_bir_lowering=False,
        debug=False,
        enable_asserts=False,
        num_devices=n_cores,
    )
    d = {
        "xT": nc.dram_tensor("xT", [C, N], MM_DT, kind="ExternalInput").ap(),
        "wqP": nc.dram_tensor("wqP", [C, PAIRS * P], MM_DT, kind="ExternalInput").ap(),
        "wkP": nc.dram_tensor("wkP", [C, PAIRS * P], MM_DT, kind="ExternalInput").ap(),
        "wvA": nc.dram_tensor("wvA", [C, VW], MM_DT, kind="ExternalInput").ap(),
        "vbA": nc.dram_tensor("vbA", [1, VW], MM_DT, kind="ExternalInput").ap(),
        "qbP": nc.dram_tensor("qbP", [P, PAIRS], F32, kind="ExternalInput").ap(),
        "kbP": nc.dram_tensor("kbP", [P, PAIRS], F32, kind="ExternalInput").ap(),
        "pwP": nc.dram_tensor("pwP", [PAIRS, P, C], MM_DT, kind="ExternalInput").ap(),
        "pbR": nc.dram_tensor("pbR", [1, C], MM_DT, kind="ExternalInput").ap(),
        "out": nc.dram_tensor("out", [N, C], F32, kind="ExternalOutput").ap(),
    }
    import contextlib

    with tile.TileContext(nc) as tc:
        with contextlib.ExitStack() as ctx:
            _emit(tc, d, ctx)
    nc.finalize()
    return nc


def _mm_np_dtype():
    if MM_DT == mybir.dt.bfloat16:
        import ml_dtypes

        return ml_dtypes.bfloat16
    return np.float32


def _prep_host(x, q_w, q_b, kv_w, kv_b, proj_w, proj_b):
    """Transpose/pack on host. Returns (per-core xT list, shared map)."""
    f32 = np.float32
    x = np.asarray(x, f32)
    xT = np.ascontiguousarray(x.reshape(B, N, C).transpose(0, 2, 1))  # [B, C, N]

    qwT = np.ascontiguousarray(np.asarray(q_w, f32).T)  # [Cin, Cout]
    kwT = np.ascontiguousarray(np.asarray(kv_w[:C], f32).T)
    vwT = np.ascontiguousarray(np.asarray(kv_w[C:], f32).T)
    pwT = np.ascontiguousarray(np.asarray(proj_w, f32).T)

    wqP = np.zeros((C, PAIRS * P), f32)
    wkP = np.zeros((C, PAIRS * P), f32)
    qbP = np.zeros((P, PAIRS), f32)
    kbP = np.zeros((P, PAIRS), f32)
    pwP = np.zeros((PAIRS, P, C), f32)
    for p in range(PAIRS):
        a, b = 2 * p, 2 * p + 1
        wqP[:, p * P : p * P + HD] = qwT[:, a * HD : (a + 1) * HD]
        wqP[:, p * P + 64 : p * P + 64 + HD] = qwT[:, b * HD : (b + 1) * HD]
        wkP[:, p * P : p * P + HD] = kwT[:, a * HD : (a + 1) * HD]
        wkP[:, p * P + 64 : p * P + 64 + HD] = kwT[:, b * HD : (b + 1) * HD]
        qbP[0:HD, p] = q_b[a * HD : (a + 1) * HD]
        qbP[64 : 64 + HD, p] = q_b[b * HD : (b + 1) * HD]
        kbP[0:HD, p] = kv_b[a * HD : (a + 1) * HD]
        kbP[64 : 64 + HD, p] = kv_b[b * HD : (b + 1) * HD]
        # rows 1..48 / 65..112 carry the proj weights; rows 0 / 64 stay zero
        # to swallow the denominator row of outT.
        pwP[p, 1 : 1 + HD, :] = pwT[a * HD : (a + 1) * HD, :]
        pwP[p, 65 : 65 + HD, :] = pwT[b * HD : (b + 1) * HD, :]

    # V blocks are [ones | v0..v47] per head so the softmax denominator lands
    # at a 32-aligned PSUM partition (0 / 64).
    wvA = np.zeros((C, VW), f32)
    vbA = np.zeros((1, VW), f32)
    for h in range(NH):
        wvA[:, h * (HD + 1) + 1 : (h + 1) * (HD + 1)] = vwT[:, h * HD : (h + 1) * HD]
        vbA[0, h * (HD + 1) + 1 : (h + 1) * (HD + 1)] = kv_b[
            C + h * HD : C + (h + 1) * HD
        ]
        vbA[0, h * (HD + 1)] = 1.0

    mmdt = _mm_np_dtype()
    shared = {
        "wqP": wqP.astype(mmdt),
        "wkP": wkP.astype(mmdt),
        "wvA": wvA.astype(mmdt),
        "vbA": vbA.astype(mmdt),
        "qbP": qbP,
        "kbP": kbP,
        "pwP": pwP.astype(mmdt),
        "pbR": np.asarray(proj_b, f32).reshape(1, C).astype(mmdt),
    }
    return xT.astype(mmdt), shared


_PROGRAM = None


def _get_program():
    global _PROGRAM
    if _PROGRAM is None:
        _PROGRAM = build_program(B)
    return _PROGRAM


def kernel(x, q_w, q_b, kv_w, kv_b, proj_w, proj_b):
    xT, shared = _prep_host(x, q_w, q_b, kv_w, kv_b, proj_w, proj_b)
    nc = _get_program()
    in_maps = [dict(shared, xT=np.ascontiguousarray(xT[b])) for b in range(B)]
    res = run_bass_kernel_spmd(nc, in_maps, list(range(B)))
    outs = [np.asarray(res.results[i]["out"], np.float32) for i in range(B)]
    return np.stack(outs).reshape(B, HH, WW, C)



# revision 33
# speedup vs baseline: 1.0211x; 1.0165x over previous
"""Trainium2 Bass kernel for nn_Attention_17532056502607.

Multi-head self-attention (B=8, N=48*48=2304 tokens, C=384, 8 heads of 48):
    q = x @ q_w.T + q_b ; k,v = x @ kv_w.T + kv_b
    out = softmax(q k^T / sqrt(48)) v ; y = out @ proj_w.T + proj_b

Sharding: data-parallel, one batch element per NeuronCore (8 cores).

Per-core algorithm (all in "S^T layout", keys on partitions — no transposes):
  - host supplies xT = x_b^T [C, N] and head-PAIR-packed weights: heads 2p /
    2p+1 of a pair live at partition rows 0-47 / 64-111, so two K=48 matmuls
    run concurrently in the PE array (row/col 32-tiles).
  - qT/kT [C_pair, N] = wT-pair @ xT          (PE, K=C=384)
  - v    [N, 8*49]    = x @ wv + rank-1 bias matmul; each head's V block is
    [ones | v0..v47], so attn@V also accumulates the softmax denominator at
    a 32-aligned output partition (0 / 64).
  - S^T  [keys, q]    = kT-tile.T @ qT        (K=48, row-packed head pairs)
  - expS = exp(scale * S^T)                   (ACT, reads PSUM, writes SBUF)
  - outT [49x2, q]   += (1|v).T @ expS        (K=128 keys, col strips 0/64)
  - normalize: drain to SBUF, exact reciprocal of rows 0/64, rank-1 selector
    matmul broadcasts the recips, DVE multiply.
  - y    [N, C]       = sum_pairs outT-pair.T @ projw-pair + bias, with K=113
    spanning both head blocks and zero weight rows under the denominators.

Matmul dtypes default to float32r for x->q/k/v and the output projection and
bf16 for the attention core (rel err ~2.7e-3 vs the fp32 reference; set
ATTN_MM_DT=float32 for exact-but-slow).
"""

import os
import sys

import numpy as np

for _p in ("/opt/trn_rl_repo",):
    if _p not in sys.path:
        sys.path.append(_p)

import concourse.bass as bass  # noqa: E402
import concourse.tile as tile  # noqa: E402
from concourse import bacc, mybir  # noqa: E402
from concourse.bass_utils import run_bass_kernel_spmd  # noqa: E402

# ---------------------------------------------------------------- constants
B = 8
HH = 48
WW = 48
C = 384
N = HH * WW  # 2304
NH = 8
HD = 48
PAIRS = NH // 2  # 4
P = 128
NT = N // P  # 18 token tiles
KTC = C // P  # 3 contraction tiles over C
SCALE = float(HD) ** -0.5
VW = NH * (HD + 1)  # 392: v with a ones column per head
CHUNKS = [(0, 512), (512, 512), (1024, 512), (1536, 512), (2048, 256)]

F32 = mybir.dt.float32
# Matmul dtype for all SBUF operands. float32 = exact but 4 cyc/row on PE;
# float32r = same bits, reduced-precision single-pass matmul (1 cyc/row for
# moving dim >= 256) but cannot write PSUM at partition base 64; bfloat16
# halves SBUF/DMA and enables FWL.
MM_DT = getattr(mybir.dt, os.environ.get("ATTN_MM_DT", "bfloat16"))
# attn@V runs in bf16 when MM_DT is float32r (f32r matmuls cannot col-tile to
# partition base 64; bf16 error here is averaged over the 2304-key softmax).
AV_DT = (
    mybir.dt.bfloat16
    if MM_DT == mybir.dt.float32r
    else getattr(mybir.dt, os.environ.get("ATTN_AV_DT", MM_DT.value))
)

# S^T (q@k) operand dtype. bf16 emits separate LDWEIGHTS instructions that
# overlap prior matmuls in the other row group (fp32r self-loads weights
# serially); the softmax averages away the extra rounding (+6% rel err).
ST_DT = getattr(
    mybir.dt,
    os.environ.get(
        "ATTN_ST_DT",
        "bfloat16" if MM_DT == mybir.dt.float32r else MM_DT.value,
    ),
)

# broadcast-matmul operand dtype: f32r is 4x faster on PE and legal at dst
# base 0; producers must write f32r-typed outputs (verifier checks rounding)
BC_DT = mybir.dt.float32r if MM_DT != mybir.dt.float32 else F32

_EXP = mybir.ActivationFunctionType.Exp


def _emit(tc: tile.TileContext, d: dict, ctx):
    nc = tc.nc

    persist = ctx.enter_context(tc.tile_pool(name="persist", bufs=1))
    v_sb = persist.tile([P, NT, VW], AV_DT, name="v_sb")
    qT_sb = persist.tile([P, PAIRS, N], ST_DT, name="qT_sb")
    kT_sb = persist.tile([P, PAIRS, N], ST_DT, name="kT_sb")
    oT_sb = persist.tile([P, PAIRS, N], MM_DT, name="oT_sb")
    pw_sb = persist.tile([P, PAIRS, C], MM_DT, name="pw_sb")
    qb_sb = persist.tile([P, PAIRS], F32, name="qb_sb")
    kb_sb = persist.tile([P, PAIRS], F32, name="kb_sb")
    vb_sb = persist.tile([1, VW], MM_DT, name="vb_sb")
    pb_sb = persist.tile([1, C], MM_DT, name="pb_sb")
    # fp32 ones vector (memset can't encode float32r); bitcast where an
    # MM_DT-typed operand is required — the bit pattern is identical.
    ones32 = persist.tile([1, P], F32, name="ones32")

    nc.vector.memset(ones32[:], 1.0)
    # zero via an F32 view: memset can't encode float32r, but 0.0 is all-zero
    # bits in every dtype
    _oT_z = oT_sb[:] if MM_DT != mybir.dt.float32r else oT_sb[:].bitcast(F32)
    nc.vector.memset(_oT_z, 0.0)
    if MM_DT == mybir.dt.bfloat16:
        ones_mm = persist.tile([1, P], MM_DT, name="ones_mm")
        nc.vector.memset(ones_mm[:], 1.0)
    elif MM_DT == mybir.dt.float32r:
        ones_mm = ones32.bitcast(MM_DT)
    else:
        ones_mm = ones32

    # SBUF inputs stay resident for the whole kernel: leftover q^T chunks are
    # computed as PE filler work woven into the attention span.
    phA = ctx.enter_context(tc.tile_pool(name="phA", bufs=1))
    fin_pool = ctx.enter_context(tc.tile_pool(name="fin", bufs=3))
    xT_sb = phA.tile([P, KTC, N], MM_DT, name="xT_sb")
    wq_sb = phA.tile([P, KTC, PAIRS * P], MM_DT, name="wq_sb")
    wk_sb = phA.tile([P, KTC, PAIRS * P], MM_DT, name="wk_sb")
    wv_sb = phA.tile([P, KTC, VW], MM_DT, name="wv_sb")
    # DMAs issued in first-consumption order (the sync engine issues them
    # serially at ~0.65us each, so order = arrival order): k weights and the
    # k^T biases first, then xT column-chunks, then q/v weights (consumed
    # ~15us into the head), and the projection weights (needed last) at the
    # very end — the strided pwP gather is the most expensive descriptor set.
    for kt in range(KTC):
        nc.sync.dma_start(
            wk_sb[:, kt, :], d["wkP"][kt * P : (kt + 1) * P, :]
        )
    nc.sync.dma_start(qb_sb[:], d["qbP"])
    nc.sync.dma_start(kb_sb[:], d["kbP"])
    for ji, (q0c, qwc) in enumerate(CHUNKS):
        for kt in range(KTC):
            nc.sync.dma_start(
                xT_sb[:, kt, q0c : q0c + qwc],
                d["xT"][kt * P : (kt + 1) * P, q0c : q0c + qwc],
            )
        if ji == 0:
            for kt in range(KTC):
                nc.sync.dma_start(
                    wq_sb[:, kt, :], d["wqP"][kt * P : (kt + 1) * P, :]
                )
        if ji == 1:
            for kt in range(KTC):
                nc.sync.dma_start(
                    wv_sb[:, kt, :], d["wvA"][kt * P : (kt + 1) * P, :]
                )
            nc.sync.dma_start(vb_sb[:], d["vbA"])
    nc.sync.dma_start(pw_sb[:], d["pwP"].rearrange("r p m -> p r m"))
    nc.sync.dma_start(pb_sb[:], d["pbR"])

    def emit_qk(pr, q0, qw, pool, tag):
        """q^T and k^T for one (pair, column chunk): pair layout, bias add."""
        psq = pool.tile([P, 512], F32, name="psq", tag=tag)
        psk = pool.tile([P, 512], F32, name="psk", tag=tag)
        for kt in range(KTC):
            nc.tensor.matmul(
                psq[:, 0:qw],
                lhsT=wq_sb[:, kt, pr * P : (pr + 1) * P],
                rhs=xT_sb[:, kt, q0 : q0 + qw],
                start=(kt == 0),
                stop=(kt == KTC - 1),
            )
            nc.tensor.matmul(
                psk[:, 0:qw],
                lhsT=wk_sb[:, kt, pr * P : (pr + 1) * P],
                rhs=xT_sb[:, kt, q0 : q0 + qw],
                start=(kt == 0),
                stop=(kt == KTC - 1),
            )
        nc.vector.tensor_scalar_add(
            qT_sb[:, pr, q0 : q0 + qw], psq[:, 0:qw], qb_sb[:, pr : pr + 1]
        )
        nc.vector.tensor_scalar_add(
            kT_sb[:, pr, q0 : q0 + qw], psk[:, 0:qw], kb_sb[:, pr : pr + 1]
        )

    def emit_k(pr, q0, qw, pool, tag):
        """k^T only (the head computes all of k first)."""
        psk = pool.tile([P, 512], F32, name="psk", tag=tag)
        for kt in range(KTC):
            nc.tensor.matmul(
                psk[:, 0:qw],
                lhsT=wk_sb[:, kt, pr * P : (pr + 1) * P],
                rhs=xT_sb[:, kt, q0 : q0 + qw],
                start=(kt == 0),
                stop=(kt == KTC - 1),
            )
        nc.vector.tensor_scalar_add(
            kT_sb[:, pr, q0 : q0 + qw], psk[:, 0:qw], kb_sb[:, pr : pr + 1]
        )

    def emit_q(pr, q0, qw, pool, tag):
        psq = pool.tile([P, 512], F32, name="psq", tag=tag)
        for kt in range(KTC):
            nc.tensor.matmul(
                psq[:, 0:qw],
                lhsT=wq_sb[:, kt, pr * P : (pr + 1) * P],
                rhs=xT_sb[:, kt, q0 : q0 + qw],
                start=(kt == 0),
                stop=(kt == KTC - 1),
            )
        nc.vector.tensor_scalar_add(
            qT_sb[:, pr, q0 : q0 + qw], psq[:, 0:qw], qb_sb[:, pr : pr + 1]
        )

    def emit_v(nt, pool, tag):
        """v natural [token, 8*(hd|1)]: K=C matmul + rank-1 (ones x vb_aug)
        which adds the v bias AND writes 1.0 into each head's 49th column."""
        psv = pool.tile([P, 512], F32, name="psv", tag=tag)
        for kt in range(KTC):
            nc.tensor.matmul(
                psv[:, 0:VW],
                lhsT=xT_sb[:, kt, nt * P : (nt + 1) * P],
                rhs=wv_sb[:, kt, :],
                start=(kt == 0),
                stop=False,
            )
        nc.tensor.matmul(
            psv[:, 0:VW], lhsT=ones_mm[:, 0:P], rhs=vb_sb[:], start=False, stop=True
        )
        nc.vector.tensor_copy(v_sb[:, nt, :], psv[:, 0:VW])

    def emit_proj(nt, pool, tag):
        """output projection for one token tile; K=113 spans both heads of a
        pair (pw rows 0, 49-63, 64 are zero; oT_sb rows 49-63 zeroed once)."""
        fF = pool.tile([P, 512], F32, name="fF", tag=tag)
        for pr in range(PAIRS):
            nc.tensor.matmul(
                fF[:, 0:C],
                lhsT=oT_sb[0:113, pr, nt * P : (nt + 1) * P],
                rhs=pw_sb[0:113, pr, :],
                start=(pr == 0),
                stop=False,
            )
        nc.tensor.matmul(
            fF[:, 0:C], lhsT=ones_mm[:, 0:P], rhs=pb_sb[:], start=False, stop=True
        )
        ft = fin_pool.tile([P, C], F32, name="ft", tag="ft")
        nc.vector.tensor_copy(ft[:], fF[:, 0:C])
        nc.sync.dma_start(d["out"][nt * P : (nt + 1) * P, :], ft[:])

    # ---------------- attention: flash, chunk-major, S^T layout ----------
    # Chunk-major (q-chunk outer, pairs inner) so each chunk's output
    # projection can run as PE filler right after its 4th pair completes,
    # instead of serializing 50us of projection after all attention.
    AV_LAG = 4  # attn@V trails exp by this many groups in steady state
    es_pool = ctx.enter_context(tc.tile_pool(name="es", bufs=8))
    rc_pool = ctx.enter_context(tc.tile_pool(name="rcp", bufs=4))
    psS = ctx.enter_context(tc.tile_pool(name="psS", bufs=1, space="PSUM"))
    # alternating 3-slot/2-slot score groups: two tags of one buf each act
    # as a double buffer in 5 banks
    GSIZES = [3, 2] * 6 + [3, 3]
    seq = [(kt, hoff) for kt in range(NT) for hoff in (0, 64)]
    # psO/psX open only after the head's PSUM pool closes (8-bank budget);
    # attn@V accumulators are therefore allocated lazily at first drain.
    psO_ref = [None]
    psX_ref = [None]
    av_q = []

    # filler queues: leftover q^T chunks (needed just-in-time) and completed
    # chunks' projections, emitted 2 per pair-block through the single spare
    # PSUM bank. Keeps the PE dense (HAM stays un-throttled) and moves the
    # projection under the ACT-bound attention span.
    qk_fill = [(c, pr) for c in range(1, len(CHUNKS)) for pr in range(PAIRS)]
    proj_fill = []

    def pop_filler(ci):
        if qk_fill and qk_fill[0][0] <= ci + 1:
            c, pr = qk_fill.pop(0)
            emit_q(pr, CHUNKS[c][0], CHUNKS[c][1], psX_ref[0], "fx")
        elif proj_fill:
            emit_proj(proj_fill.pop(0), psX_ref[0], "fx")
        elif qk_fill:
            c, pr = qk_fill.pop(0)
            emit_q(pr, CHUNKS[c][0], CHUNKS[c][1], psX_ref[0], "fx")

    def attnv(est, si, gs, blk):
        if blk["oTP"] is None:
            blk["oTP"] = psO_ref[0].tile([P, 512], F32, name="oTP", tag="oTP")
        for j in range(gs):
            kt2, hoff2 = seq[si + j]
            h = blk["pr"] * 2 + (0 if hoff2 == 0 else 1)
            nc.tensor.matmul(
                blk["oTP"][hoff2 : hoff2 + HD + 1, 0 : blk["qw"]],
                lhsT=v_sb[:, kt2, h * (HD + 1) : (h + 1) * (HD + 1)],
                rhs=est[:, j, 0 : blk["qw"]],
                start=(kt2 == 0),
                stop=(kt2 == NT - 1),
            )
        if si + gs == 2 * NT:
            normalize(blk)

    def emit_stq_group(blk, gi, gs, si):
        """One score group: S^T matmuls into PSUM, exp to SBUF, queue attn@V."""
        pr, q0, qw = blk["pr"], blk["q0"], blk["qw"]
        sg = psS.tile([P, gs, 512], F32, name="sg", tag=f"sg{gs}")
        for j in range(gs):
            kt, hoff = seq[si + j]
            nc.tensor.matmul(
                sg[:, j, 0:qw],
                lhsT=kT_sb[hoff : hoff + HD, pr, kt * P : (kt + 1) * P],
                rhs=qT_sb[hoff : hoff + HD, pr, q0 : q0 + qw],
                start=True,
                stop=True,
            )
        est = es_pool.tile([P, gs, 512], AV_DT, name="est", tag=f"est{gs}")
        nc.scalar.activation(est[:, :, 0:qw], sg[:, :, 0:qw], _EXP, scale=SCALE)
        av_q.append((est, si, gs, blk))

    def normalize(blk):
        # ~51-ULP approx reciprocals straight off the two PSUM
        # denominator rows (0 / 64), both landing at a base-0 SBUF row
        # (cross-base is legal for single-input ops when both bases are
        # 32-aligned). GPSIMD (otherwise idle) broadcasts them across
        # partitions: recA over rows 0-48, recB over rows 0-112 so the
        # head-B multiply can read the 64:113 slice with matching input
        # bases. The normalize is two DVE multiplies reading the
        # accumulator PSUM directly — no PE work, no drain copies.
        oTP, pr, q0, qw = blk["oTP"], blk["pr"], blk["q0"], blk["qw"]
        recA = rc_pool.tile([1, 512], F32, name="recA", tag="rcA")
        recB = rc_pool.tile([1, 512], F32, name="recB", tag="rcB")
        nc.vector.reciprocal_approx_fast(recA[0:1, 0:qw], oTP[0:1, 0:qw])
        nc.vector.reciprocal_approx_fast(recB[0:1, 0:qw], oTP[64:65, 0:qw])
        bcsA = rc_pool.tile([P, 512], F32, name="bcsA", tag="bcsA")
        bcsB = rc_pool.tile([P, 512], F32, name="bcsB", tag="bcsB")
        nc.gpsimd.partition_broadcast(bcsA[0 : HD + 1, 0:qw], recA[0:1, 0:qw])
        nc.gpsimd.partition_broadcast(
            bcsB[0 : 64 + HD + 1, 0:qw], recB[0:1, 0:qw]
        )
        nc.vector.tensor_mul(
            oT_sb[0 : HD + 1, pr, q0 : q0 + qw],
            oTP[0 : HD + 1, 0:qw],
            bcsA[0 : HD + 1, 0:qw],
        )
        nc.vector.tensor_mul(
            oT_sb[64 : 64 + HD + 1, pr, q0 : q0 + qw],
            oTP[64 : 64 + HD + 1, 0:qw],
            bcsB[64 : 64 + HD + 1, 0:qw],
        )
        # once a chunk's 4th pair is normalized, its token tiles are
        # fully attended — queue their output projections as filler
        if pr == PAIRS - 1:
            for nt in range(q0 // P, (q0 + qw) // P):
                proj_fill.append(nt)

    # ---------------- head, overlapped with the first attention block -----
    # Only pair 0's k^T and chunk-0 q^T gate the first scores. The rest of
    # the head (k^T for pairs 1-3, chunk-0 q^T, all of v) interleaves with
    # block (c0,p0)'s S^T+exp groups, 3 units per group, through a 3-bank
    # PSUM pool (3 + sg's 5 = 8). attn@V for block 0 is fully deferred —
    # its accumulator pool can only open once the head pool closes — and
    # catches up during block (c0,p1), at most 2 drains per group.
    with tc.tile_pool(name="psH", bufs=3, space="PSUM") as psH:
        for q0h, qwh in CHUNKS:
            emit_k(0, q0h, qwh, psH, "ph")
        emit_q(0, CHUNKS[0][0], CHUNKS[0][1], psH, "ph")

        head_units = []
        for pr in range(1, PAIRS):
            for q0h, qwh in CHUNKS:
                head_units.append(
                    lambda pr=pr, q0h=q0h, qwh=qwh: emit_k(pr, q0h, qwh, psH, "ph")
                )
            head_units.append(
                lambda pr=pr: emit_q(pr, CHUNKS[0][0], CHUNKS[0][1], psH, "ph")
            )
        for nt in range(NT):
            head_units.append(lambda nt=nt: emit_v(nt, psH, "ph"))

        blk0 = {"oTP": None, "pr": 0, "q0": CHUNKS[0][0], "qw": CHUNKS[0][1]}
        si = 0
        for gi, gs in enumerate(GSIZES):
            for _ in range(3):
                if head_units:
                    head_units.pop(0)()
            emit_stq_group(blk0, gi, gs, si)
            si += gs
        while head_units:
            head_units.pop(0)()

    with (
        tc.tile_pool(name="psO", bufs=2, space="PSUM") as psO,
        tc.tile_pool(name="psX", bufs=1, space="PSUM") as psX,
    ):
        psO_ref[0] = psO
        psX_ref[0] = psX
        # The attn@V queue carries ACROSS pair-blocks: a block's tail
        # attn@V matmuls interleave with the next block's first S^T groups
        # instead of flushing serially at the boundary (which parked the
        # in-order PE queue and starved ACT for ~2.5us per block).
        for cidx, (q0, qw) in enumerate(CHUNKS):
            # correctness: this chunk's q^T must be emitted before any S^T
            # references it
            while qk_fill and qk_fill[0][0] <= cidx:
                c, pr = qk_fill.pop(0)
                emit_q(pr, CHUNKS[c][0], CHUNKS[c][1], psX, "fx")
            for pr in range(PAIRS):
                if cidx == 0 and pr == 0:
                    continue  # emitted inside the head overlap above
                # one accumulator bank for both heads: head A at partitions
                # 0-48 (col strips 0-1), head B at 64-112 (strips 2-3) — the
                # two matmuls stay concurrent via col tiling, and bufs=2
                # double-buffers the bank across blocks so this block's
                # attn@V never waits on the previous block's normalize.
                blk = {"oTP": None, "pr": pr, "q0": q0, "qw": qw}
                si = 0
                # more filler slots in the last chunks so the remaining
                # projections drain before the attention stream ends
                slots = (4, 10) if cidx < 3 else (2, 6, 10)
                for gi, gs in enumerate(GSIZES):
                    if gi in slots:
                        pop_filler(cidx)
                    emit_stq_group(blk, gi, gs, si)
                    pops = 0
                    while len(av_q) > AV_LAG and pops < 2:
                        attnv(*av_q.pop(0))
                        pops += 1
                    si += gs

        # drain the attn@V tail, then whatever filler work remains
        for av in av_q:
            attnv(*av)
        av_q.clear()
        while qk_fill or proj_fill:
            pop_filler(len(CHUNKS))


def build_program(n_cores: int = 8):
    nc = bacc.Bacc(
        "TRN2",
        target_bir_lowering=False,
        debug=False,
        enable_asserts=False,
        num_devices=n_cores,
    )
    d = {
        "xT": nc.dram_tensor("xT", [C, N], MM_DT, kind="ExternalInput").ap(),
        "wqP": nc.dram_tensor("wqP", [C, PAIRS * P], MM_DT, kind="ExternalInput").ap(),
        "wkP": nc.dram_tensor("wkP", [C, PAIRS * P], MM_DT, kind="ExternalInput").ap(),
        "wvA": nc.dram_tensor("wvA", [C, VW], MM_DT, kind="ExternalInput").ap(),
        "vbA": nc.dram_tensor("vbA", [1, VW], MM_DT, kind="ExternalInput").ap(),
        "qbP": nc.dram_tensor("qbP", [P, PAIRS], F32, kind="ExternalInput").ap(),
        "kbP": nc.dram_tensor("kbP", [P, PAIRS], F32, kind="ExternalInput").ap(),
        "pwP": nc.dram_tensor("pwP", [PAIRS, P, C], MM_DT, kind="ExternalInput").ap(),
        "pbR": nc.dram_tensor("pbR", [1, C], MM_DT, kind="ExternalInput").ap(),
        "out": nc.dram_tensor("out", [N, C], F32, kind="ExternalOutput").ap(),
    }
    import contextlib

    with tile.TileContext(nc) as tc:
        with contextlib.ExitStack() as ctx:
            _emit(tc, d, ctx)
    nc.finalize()
    return nc


def _mm_np_dtype():
    if MM_DT == mybir.dt.bfloat16:
        import ml_dtypes

        return ml_dtypes.bfloat16
    return np.float32


def _prep_host(x, q_w, q_b, kv_w, kv_b, proj_w, proj_b):
    """Transpose/pack on host. Returns (per-core xT list, shared map)."""
    f32 = np.float32
    x = np.asarray(x, f32)
    xT = np.ascontiguousarray(x.reshape(B, N, C).transpose(0, 2, 1))  # [B, C, N]

    qwT = np.ascontiguousarray(np.asarray(q_w, f32).T)  # [Cin, Cout]
    kwT = np.ascontiguousarray(np.asarray(kv_w[:C], f32).T)
    vwT = np.ascontiguousarray(np.asarray(kv_w[C:], f32).T)
    pwT = np.ascontiguousarray(np.asarray(proj_w, f32).T)

    wqP = np.zeros((C, PAIRS * P), f32)
    wkP = np.zeros((C, PAIRS * P), f32)
    qbP = np.zeros((P, PAIRS), f32)
    kbP = np.zeros((P, PAIRS), f32)
    pwP = np.zeros((PAIRS, P, C), f32)
    for p in range(PAIRS):
        a, b = 2 * p, 2 * p + 1
        wqP[:, p * P : p * P + HD] = qwT[:, a * HD : (a + 1) * HD]
        wqP[:, p * P + 64 : p * P + 64 + HD] = qwT[:, b * HD : (b + 1) * HD]
        wkP[:, p * P : p * P + HD] = kwT[:, a * HD : (a + 1) * HD]
        wkP[:, p * P + 64 : p * P + 64 + HD] = kwT[:, b * HD : (b + 1) * HD]
        qbP[0:HD, p] = q_b[a * HD : (a + 1) * HD]
        qbP[64 : 64 + HD, p] = q_b[b * HD : (b + 1) * HD]
        kbP[0:HD, p] = kv_b[a * HD : (a + 1) * HD]
        kbP[64 : 64 + HD, p] = kv_b[b * HD : (b + 1) * HD]
        # rows 1..48 / 65..112 carry the proj weights; rows 0 / 64 stay zero
        # to swallow the denominator row of outT.
        pwP[p, 1 : 1 + HD, :] = pwT[a * HD : (a + 1) * HD, :]
        pwP[p, 65 : 65 + HD, :] = pwT[b * HD : (b + 1) * HD, :]

    # V blocks are [ones | v0..v47] per head so the softmax denominator lands
    # at a 32-aligned PSUM partition (0 / 64).
    wvA = np.zeros((C, VW), f32)
    vbA = np.zeros((1, VW), f32)
    for h in range(NH):
        wvA[:, h * (HD + 1) + 1 : (h + 1) * (HD + 1)] = vwT[:, h * HD : (h + 1) * HD]
        vbA[0, h * (HD + 1) + 1 : (h + 1) * (HD + 1)] = kv_b[
            C + h * HD : C + (h + 1) * HD
        ]
        vbA[0, h * (HD + 1)] = 1.0

    mmdt = _mm_np_dtype()
    shared = {
        "wqP": wqP.astype(mmdt),
        "wkP": wkP.astype(mmdt),
        "wvA": wvA.astype(mmdt),
        "vbA": vbA.astype(mmdt),
        "qbP": qbP,
        "kbP": kbP,
        "pwP": pwP.astype(mmdt),
        "pbR": np.asarray(proj_b, f32).reshape(1, C).astype(mmdt),
    }
    return xT.astype(mmdt), shared


_PROGRAM = None


def _get_program():
    global _PROGRAM
    if _PROGRAM is None:
        _PROGRAM = build_program(B)
    return _PROGRAM


def kernel(x, q_w, q_b, kv_w, kv_b, proj_w, proj_b):
    xT, shared = _prep_host(x, q_w, q_b, kv_w, kv_b, proj_w, proj_b)
    nc = _get_program()
    in_maps = [dict(shared, xT=np.ascontiguousarray(xT[b])) for b in range(B)]
    res = run_bass_kernel_spmd(nc, in_maps, list(range(B)))
    outs = [np.asarray(res.results[i]["out"], np.float32) for i in range(B)]
    return np.stack(outs).reshape(B, HH, WW, C)

